# revision 22
# baseline (speedup 1.0000x reference)
"""Talking-heads attention (ViT-B/16-ish shapes) on 8 Trainium2 NeuronCores.

Problem: B=16, N=577, C=768, H=12 heads, d=64.
  qkv = x @ Wqkv.T ; logits = q k^T * scale ; pre-softmax head mix (Wpre);
  softmax ; post-softmax head mix (Wpost) ; out = (attn @ v) @ Wproj.T + b.

Distribution: pure data-parallel over batch, 2 batches per core, no
collectives.

Per-core design (all matmuls bf16 inputs, fp32 PSUM accumulation):
  - host pre-transposes x to [C, N] and pre-casts/packs all weights.
  - qkv:   q,k in [feat, tok] layout; v in [tok, feat] layout.
  - logits per head, K=64, two heads run concurrently via PE row groups.
  - talking-heads mixing runs as 120x120 block-diagonal matmuls in a packed
    layout [(h-major: p = 10h + n_i), m] over blocks of 10 query rows.
    The partition interleave that builds this layout round-trips through a
    DRAM scratch laid out [h][ni][block][m_pad]: 12 block writes per qtile
    (1168B-aligned runs) + ONE fully-contiguous packed read per qtile
    ([120 partitions x 14KB runs]), all on otherwise-idle DMA queues.
  - softmax without max-subtraction (logits are small); exp on ScalarE with
    accum_out producing the row sums.  The 1/rowsum normalization is folded
    into the tiny 120x120 post-mix matrix (scale its rows per-partition)
    instead of rescaling the big [120, 577] exp tensor.
  - post-mix is fused with the transpose AV needs: E-tile is the stationary
    operand, the sinv-scaled block-diag Wpost^T the moving one, giving
    P'^T[m, (10g+n)] in PSUM directly.
  - AV consumes P'^T with a strided free AP per head; head pairs run
    concurrently via PE column groups. Output lands in [feat, tok] layout,
    which feeds the final projection without any transpose.
  - emission is block-interleaved (postmix/AV/proj of qtile j alongside
    logits of qtile j+2 and premix of j+1) over one shared 4-deep PSUM pool
    so the PE always has queued work and the HAM clock gate stays warm.
"""

import numpy as np
import ml_dtypes

import concourse.bass as bass
import concourse.mybir as mybir
from concourse import bacc
from concourse.tile import TileContext
from concourse.bass_utils import run_bass_kernel_spmd

BF16 = ml_dtypes.bfloat16

B, N, C, H = 16, 577, 768, 12
D = C // H                 # 64
NCORES = 8
BPC = B // NCORES          # batches per core = 2
NPAD = 600                 # padded query-token count (5 qtiles of 120)
QT = 5                     # query tiles per batch
NJ = BPC * QT              # total qtile jobs per core = 10
QTW = 120                  # rows per query tile
NI = 10                    # query rows per packed block
BPQ = QTW // NI            # blocks per qtile = 12
FT = C // 128              # feature tiles = 6
MT = [128, 128, 128, 128, 65]   # key-token tiles (sum 577)
MOF = [0, 128, 256, 384, 512]
MP = 584                   # padded key count for the pack scratch (16B runs)

_NC_CACHE = {}


def _build_nc():
    nc = bacc.Bacc("TRN2", target_bir_lowering=False)
    dt = mybir.dt

    xT = nc.dram_tensor("xT", [BPC, C, NPAD], dt.bfloat16, kind="ExternalInput")
    wqT = nc.dram_tensor("wqT", [C, C], dt.bfloat16, kind="ExternalInput")
    wkT = nc.dram_tensor("wkT", [C, C], dt.bfloat16, kind="ExternalInput")
    wvT = nc.dram_tensor("wvT", [C, C], dt.bfloat16, kind="ExternalInput")
    wpT = nc.dram_tensor("wpT", [C, C], dt.bfloat16, kind="ExternalInput")
    bdpre = nc.dram_tensor("bdpre", [QTW, QTW], dt.bfloat16, kind="ExternalInput")
    bdpostT = nc.dram_tensor("bdpostT", [QTW, QTW], dt.bfloat16, kind="ExternalInput")
    bias = nc.dram_tensor("bias", [C], dt.float32, kind="ExternalInput")
    y = nc.dram_tensor("y", [BPC, N, C], dt.float32, kind="ExternalOutput")
    # packed-logits scratch: [batch][qtile][h][ni][block][m] so that the
    # packed read is one [120 part x 14KB contiguous] DMA per qtile.
    pk = nc.dram_tensor("pk", [BPC, QT, H, NI, BPQ, MP], dt.bfloat16,
                        kind="Internal")

    with TileContext(nc) as tc:
        with (
            tc.tile_pool(name="consts", bufs=1) as consts,
            tc.tile_pool(name="io1", bufs=1) as io1,        # xT, q, k, o
            tc.tile_pool(name="vpool", bufs=2) as vpool,    # v (alive across batch overlap)
            tc.tile_pool(name="lnatp", bufs=2) as lnatp,
            tc.tile_pool(name="lpkp", bufs=2) as lpkp,
            tc.tile_pool(name="ep", bufs=2) as ep,
            tc.tile_pool(name="ptp", bufs=1) as ptp,
            tc.tile_pool(name="stage", bufs=2) as stage,
            tc.tile_pool(name="outp", bufs=2) as outp,
            tc.tile_pool(name="psp", bufs=2, space="PSUM") as psp,
            tc.tile_pool(name="psq", bufs=4, space="PSUM") as psq,
        ):
            # ---- constants (xT of batch 0 + q/k weights first) ----
            wq_sb = consts.tile([128, FT, C], dt.bfloat16, tag="wq")
            wk_sb = consts.tile([128, FT, C], dt.bfloat16, tag="wk")
            wv_sb = consts.tile([128, FT, C], dt.bfloat16, tag="wv")
            wp_sb = consts.tile([128, FT, C], dt.bfloat16, tag="wp")
            for w_sb, w_dr, eng in ((wq_sb, wqT, nc.scalar), (wk_sb, wkT, nc.gpsimd),
                                    (wv_sb, wvT, nc.scalar), (wp_sb, wpT, nc.gpsimd)):
                eng.dma_start(out=w_sb[:], in_=w_dr.rearrange("(t p) f -> p t f", p=128))
            bdpre_sb = consts.tile([QTW, QTW], dt.bfloat16, tag="bdpre")
            nc.scalar.dma_start(out=bdpre_sb[:], in_=bdpre[:])
            bdpostT_sb = consts.tile([QTW, QTW], dt.bfloat16, tag="bdpostT")
            nc.gpsimd.dma_start(out=bdpostT_sb[:], in_=bdpostT[:])
            bias_sb = consts.tile([128, C], dt.float32, tag="bias")
            nc.scalar.dma_start(
                out=bias_sb[:],
                in_=bass.AP(tensor=bias[:].tensor, offset=0, ap=[[0, 128], [1, C]]),
            )

            # per-batch SBUF state, filled by emit_load/emit_qkv
            st = [dict() for _ in range(BPC)]

            def emit_load(bi):
                xT_sb = io1.tile([128, FT, NPAD], dt.bfloat16, tag="xT")
                nc.sync.dma_start(
                    out=xT_sb[:], in_=xT[bi].rearrange("(t p) n -> p t n", p=128)
                )
                st[bi]["xT"] = xT_sb

            def qkv_units(bi, which="all"):
                """thunks for qkv stage; which in ('qk', 'v', 'all').
                q/k units must precede logits; v is only needed by AV."""
                xT_sb = st[bi]["xT"]
                if "q" not in st[bi]:
                    q_sb = io1.tile([128, FT, NPAD], dt.bfloat16, tag="q")
                    k_sb = io1.tile([128, FT, MP], dt.bfloat16, tag="k")
                    v_sb = vpool.tile([128, len(MT), C], dt.bfloat16, tag="v")
                    st[bi].update(q=q_sb, k=k_sb, v=v_sb)
                q_sb, k_sb, v_sb = st[bi]["q"], st[bi]["k"], st[bi]["v"]
                units = []
                uidx = [0]

                def pad_unit():
                    # zero k's pad columns so padded logits are exactly 0
                    nc.vector.memset(k_sb[:, :, N:MP], 0.0)

                def qk_unit(ft, which):
                    def emit():
                        dst, w_sb, ntok = (
                            (q_sb, wq_sb, NPAD) if which == 0 else (k_sb, wk_sb, N)
                        )
                        for lo, hi in ((0, 512), (512, ntok)):
                            ps = psq.tile([128, 512], dt.float32, tag="psq")
                            for kc in range(FT):
                                nc.tensor.matmul(
                                    out=ps[:, 0:hi - lo],
                                    lhsT=w_sb[:, kc, ft * 128:(ft + 1) * 128],
                                    rhs=xT_sb[:, kc, lo:hi],
                                    start=(kc == 0), stop=(kc == FT - 1),
                                )
                            if uidx[0] % 2 == 0:
                                nc.vector.tensor_copy(out=dst[:, ft, lo:hi],
                                                      in_=ps[:, 0:hi - lo])
                            else:
                                nc.scalar.copy(out=dst[:, ft, lo:hi],
                                               in_=ps[:, 0:hi - lo])
                            uidx[0] += 1
                    return emit

                def v_unit(mt):
                    def emit():
                        mw = MT[mt]
                        for lo, hi in ((0, 512), (512, C)):
                            ps = psq.tile([128, 512], dt.float32, tag="psq")
                            for kc in range(FT):
                                nc.tensor.matmul(
                                    out=ps[0:mw, 0:hi - lo],
                                    lhsT=xT_sb[:, kc, MOF[mt]:MOF[mt] + mw],
                                    rhs=wv_sb[:, kc, lo:hi],
                                    start=(kc == 0), stop=(kc == FT - 1),
                                )
                            if uidx[0] % 2 == 0:
                                nc.vector.tensor_copy(out=v_sb[0:mw, mt, lo:hi],
                                                      in_=ps[0:mw, 0:hi - lo])
                            else:
                                nc.scalar.copy(out=v_sb[0:mw, mt, lo:hi],
                                               in_=ps[0:mw, 0:hi - lo])
                            uidx[0] += 1
                    return emit

                if which in ("qk", "all"):
                    units.append(pad_unit)
                    for ft in range(FT):
                        units.append(qk_unit(ft, 0))
                        units.append(qk_unit(ft, 1))
                if which in ("v", "all"):
                    for mt in range(len(MT)):
                        units.append(v_unit(mt))
                return units

            # ---- per-job state (j = bi*QT + qt) ----
            lnats = {}
            lpks = {}
            es = {}
            ss = {}
            bdscs = {}
            pts = {}

            def L_units(j):
                """logits of job j: 6 head-pair units writing l_nat."""
                bi, qt = divmod(j, QT)
                q0 = qt * QTW
                q_sb, k_sb = st[bi]["q"], st[bi]["k"]
                l_nat = lnatp.tile([QTW, H, MP], dt.bfloat16, tag="lnat")
                lnats[j] = l_nat
                units = []

                def hp_unit(hp):
                    def emit():
                        ps0 = psp.tile([QTW, MP], dt.float32, tag="ps")
                        ps1 = psp.tile([QTW, MP], dt.float32, tag="ps")
                        for sub, ps in ((0, ps0), (1, ps1)):
                            pbase = 64 * sub
                            for lo, hi in ((0, 512), (512, MP)):
                                nc.tensor.matmul(
                                    out=ps[:, lo:hi],
                                    lhsT=q_sb[pbase:pbase + 64, hp, q0:q0 + QTW],
                                    rhs=k_sb[pbase:pbase + 64, hp, lo:hi],
                                )
                        for sub, ps in ((0, ps0), (1, ps1)):
                            idx = 2 * hp + sub
                            dst = l_nat[:, idx, :]
                            if idx in (3, 7, 11):
                                nc.scalar.copy(out=dst, in_=ps[:])
                            else:
                                nc.vector.tensor_copy(out=dst, in_=ps[:])
                            # per-head pack write, gated only on this evac:
                            # overlaps the rest of the L stage.
                            nc.gpsimd.dma_start(
                                out=pk[bi, qt, idx].rearrange("ni b m -> b ni m"),
                                in_=l_nat[:, idx, :],
                            )
                    return emit

                for hp in range(H // 2):
                    units.append(hp_unit(hp))
                return units

            def emit_read(j):
                bi, qt = divmod(j, QT)
                l_pk = lpkp.tile([QTW, BPQ, MP], dt.bfloat16, tag="lpk")
                lpks[j] = l_pk
                hb = BPQ // 2
                for b0 in (0, hb):
                    nc.sync.dma_start(
                        out=l_pk[:, b0:b0 + hb, :].rearrange("p b m -> p (b m)"),
                        in_=pk[bi, qt, :, :, b0:b0 + hb, :].rearrange(
                            "h ni b m -> (h ni) (b m)"),
                    )

            def P_units(j):
                """premix + exp of job j (needs lpks[j]); then recip + bdscale."""
                l_pk = lpks[j]
                e_sb = ep.tile([QTW, BPQ, MP], dt.bfloat16, tag="e")
                s_sb = stage.tile([QTW, BPQ], dt.float32, tag="s")
                es[j], ss[j] = e_sb, s_sb
                units = []

                def pm_unit(b):
                    def emit():
                        ps = psp.tile([QTW, MP], dt.float32, tag="ps")
                        for lo, hi in ((0, 512), (512, N)):
                            nc.tensor.matmul(
                                out=ps[:, lo:hi], lhsT=bdpre_sb[:], rhs=l_pk[:, b, lo:hi]
                            )
                        nc.scalar.activation(
                            out=e_sb[:, b, 0:N], in_=ps[:, 0:N],
                            func=mybir.ActivationFunctionType.Exp,
                            accum_out=s_sb[:, b:b + 1],
                        )
                    return emit

                sinv = stage.tile([QTW, BPQ], dt.float32, tag="sinv")
                bdsc = stage.tile([QTW, BPQ, QTW], dt.bfloat16, tag="bdsc")
                bdscs[j] = bdsc
                hb = BPQ // 2

                def scale_unit(half):
                    def emit():
                        # per-half so the first postmix groups don't wait for
                        # the last exps of this job
                        lo = half * hb
                        nc.vector.reciprocal(
                            out=sinv[:, lo:lo + hb], in_=s_sb[:, lo:lo + hb])
                        for b in range(lo, lo + hb):
                            # gpsimd is idle outside DMA issue; offload the
                            # tiny per-block matrix scaling from the DVE
                            nc.gpsimd.tensor_scalar_mul(
                                bdsc[:, b, :], bdpostT_sb[:], sinv[:, b:b + 1]
                            )
                    return emit

                for b in range(BPQ):
                    units.append(pm_unit(b))
                    if b == hb - 1:
                        units.append(scale_unit(0))
                units.append(scale_unit(1))
                return units

            def Q1_units(j):
                """postmix (fused with transpose): 15 groups of 4 blocks."""
                e_sb = es[j]
                pt_sb = ptp.tile([128, len(MT), BPQ, QTW], dt.bfloat16, tag="pt")
                pts[j] = pt_sb
                bdsc = bdscs[j]
                units = []

                def grp_unit(mt, bg, idx):
                    def emit():
                        mw = MT[mt]
                        ps = psq.tile([128, 4 * QTW], dt.float32, tag="psq")
                        for sl in range(4):
                            b = 4 * bg + sl
                            nc.tensor.matmul(
                                out=ps[0:mw, sl * QTW:(sl + 1) * QTW],
                                lhsT=e_sb[:, b, MOF[mt]:MOF[mt] + mw],
                                rhs=bdsc[:, b, :],
                            )
                        dst = pt_sb[0:mw, mt, 4 * bg:4 * (bg + 1), :]
                        if idx % 2 == 0:
                            nc.scalar.copy(out=dst, in_=ps[0:mw, 0:4 * QTW])
                        else:
                            nc.vector.tensor_copy(out=dst, in_=ps[0:mw, 0:4 * QTW])
                    return emit

                # bg-major: the first groups only need the first half of bdsc
                idx = 0
                for bg in range(BPQ // 4):
                    for mt in range(len(MT)):
                        units.append(grp_unit(mt, bg, idx))
                        idx += 1
                return units

            def get_o(bi):
                # allocated lazily: first writer is AV of (bi, qt0), which on
                # the PE stream comes after proj of (bi-1, qt4) — the last
                # reader of the previous batch's o — keeping bufs=1 feasible.
                if "o" not in st[bi]:
                    o_sb = io1.tile([128, FT, NPAD], dt.bfloat16, tag="o")
                    st[bi]["o"] = o_sb
                return st[bi]["o"]

            def Q2_units(j):
                """AV: head pairs via PE column groups, into o[feat, tok]."""
                bi, qt = divmod(j, QT)
                q0 = qt * QTW
                v_sb, o_sb = st[bi]["v"], get_o(bi)
                pt_sb = pts[j]
                units = []

                def gpp_unit(a):
                    def emit():
                        # two head-pairs (4 heads) share one 1-bank PSUM tile
                        ps = psq.tile([128, 2 * QTW], dt.float32, tag="psq")
                        for half in range(2):
                            gp = 2 * a + half
                            for sub in range(2):
                                g = 2 * gp + sub
                                for mt in range(len(MT)):
                                    mw = MT[mt]
                                    nc.tensor.matmul(
                                        out=ps[64 * sub:64 * (sub + 1),
                                               half * QTW:(half + 1) * QTW],
                                        lhsT=v_sb[0:mw, mt, 64 * g:64 * (g + 1)],
                                        rhs=pt_sb[0:mw, mt, :, NI * g:NI * (g + 1)],
                                        start=(mt == 0), stop=(mt == len(MT) - 1),
                                        skip_group_check=True,
                                    )
                        nc.vector.tensor_copy(
                            out=o_sb[:, 2 * a:2 * a + 2, q0:q0 + QTW],
                            in_=ps[:].rearrange("p (h n) -> p h n", h=2),
                        )
                    return emit

                for a in range(H // 4):
                    units.append(gpp_unit(a))
                return units

            def Q3_unit(j):
                """output projection + bias + y write for job j."""
                bi, qt = divmod(j, QT)
                q0 = qt * QTW
                o_sb = get_o(bi)
                out_sb = outp.tile([QTW, C], dt.float32, tag="out")
                for lo, hi in ((0, 512), (512, C)):
                    ps = psq.tile([QTW, 512], dt.float32, tag="psq")
                    for kc in range(FT):
                        nc.tensor.matmul(
                            out=ps[:, 0:hi - lo],
                            lhsT=o_sb[:, kc, q0:q0 + QTW],
                            rhs=wp_sb[:, kc, lo:hi],
                            start=(kc == 0), stop=(kc == FT - 1),
                        )
                    nc.vector.tensor_tensor(
                        out=out_sb[:, lo:hi], in0=ps[:, 0:hi - lo],
                        in1=bias_sb[0:QTW, lo:hi], op=mybir.AluOpType.add,
                    )
                rows = min(N - q0, QTW)
                nc.gpsimd.dma_start(out=y[bi, q0:q0 + rows, :], in_=out_sb[0:rows, :])

            # ================= global schedule =================
            # prologue: q/k first, logits of job 0 ASAP (its pack round-trip
            # is on the critical path to the first premix); v afterwards.
            emit_load(0)
            for u in qkv_units(0, "qk"):
                u()
            for u in L_units(0):
                u()
            emit_read(0)
            for u in qkv_units(0, "v"):
                u()
            for u in L_units(1):
                u()
            emit_read(1)
            for u in P_units(0):
                u()

            for j in range(NJ):
                # L(j+2) emitted densely at block start so its evacs + pack
                # round-trip complete ~1.5 blocks before premix(j+2) needs it.
                if j == 1:
                    emit_load(1)
                if j + 2 < NJ:
                    for u in L_units(j + 2):
                        u()
                    emit_read(j + 2)
                if j == 2:
                    # batch-1 qkv AFTER L(4): L(4) is the last reader of
                    # batch-0's q/k, whose slots (bufs=1) batch 1 reuses.
                    for u in qkv_units(1, "all"):
                        u()
                for u in Q1_units(j):
                    u()
                for u in Q2_units(j):
                    u()
                Q3_unit(j)
                if j + 1 < NJ:
                    for u in P_units(j + 1):
                        u()
                # release dead per-job references
                for dd in (lnats, lpks, es, ss, bdscs, pts):
                    dd.pop(j, None)
    nc.compile()
    return nc


def _host_prep(x, Wqkv, Wproj, bproj, Wpre, Wpost):
    scale = D ** -0.5
    Wq = (Wqkv[0:C] * scale).T        # [C, C] lhsT for q (scale folded)
    Wk = Wqkv[C:2 * C].T
    Wv = Wqkv[2 * C:3 * C].T
    Wp = Wproj.T
    # h-major packed-block mixing matrices (p = 10*h + n_i)
    eye = np.eye(NI, dtype=np.float32)
    # bdpre[(10h+ni), (10g+nj)] = Wpre[g, h] * (ni == nj)
    bdpre = np.einsum("gh,ij->higj", Wpre.astype(np.float32), eye).reshape(QTW, QTW)
    # bdpostT[(10g+ni), (10g'+nj)] = Wpost[g', g] * (ni == nj)
    bdpostT = np.einsum("pg,ij->gipj", Wpost.astype(np.float32), eye).reshape(QTW, QTW)

    xT = np.zeros((B, C, NPAD), dtype=BF16)
    xT[:, :, 0:N] = np.ascontiguousarray(x.transpose(0, 2, 1)).astype(BF16)
    return {
        "xT": xT,
        "wqT": np.ascontiguousarray(Wq).astype(BF16),
        "wkT": np.ascontiguousarray(Wk).astype(BF16),
        "wvT": np.ascontiguousarray(Wv).astype(BF16),
        "wpT": np.ascontiguousarray(Wp).astype(BF16),
        "bdpre": bdpre.astype(BF16),
        "bdpostT": bdpostT.astype(BF16),
        "bias": bproj.astype(np.float32),
    }


def kernel(x, Wqkv, Wproj, bproj, Wpre, Wpost):
    x = np.asarray(x, dtype=np.float32)
    Wqkv = np.asarray(Wqkv, dtype=np.float32)
    Wproj = np.asarray(Wproj, dtype=np.float32)
    bproj = np.asarray(bproj, dtype=np.float32)
    Wpre = np.asarray(Wpre, dtype=np.float32)
    Wpost = np.asarray(Wpost, dtype=np.float32)

    host = _host_prep(x, Wqkv, Wproj, bproj, Wpre, Wpost)
    if "nc" not in _NC_CACHE:
        _NC_CACHE["nc"] = _build_nc()
    nc = _NC_CACHE["nc"]

    shared = {k: host[k] for k in
              ("wqT", "wkT", "wvT", "wpT", "bdpre", "bdpostT", "bias")}
    in_maps = []
    for core in range(NCORES):
        m = dict(shared)
        m["xT"] = host["xT"][core * BPC:(core + 1) * BPC]
        in_maps.append(m)

    res = run_bass_kernel_spmd(nc, in_maps, core_ids=list(range(NCORES)))
    out = np.concatenate([np.asarray(r["y"]) for r in res.results], axis=0)
    return out.astype(np.float32)


# revision 24
# speedup vs baseline: 1.0447x; 1.0447x over previous
"""Talking-heads attention (ViT-B/16-ish shapes) on 8 Trainium2 NeuronCores.

Problem: B=16, N=577, C=768, H=12 heads, d=64.
  qkv = x @ Wqkv.T ; logits = q k^T * scale ; pre-softmax head mix (Wpre);
  softmax ; post-softmax head mix (Wpost) ; out = (attn @ v) @ Wproj.T + b.

Distribution: pure data-parallel over batch, 2 batches per core, no
collectives.

Per-core design (all matmuls bf16 inputs, fp32 PSUM accumulation):
  - host pre-transposes x to [C, N] and pre-casts/packs all weights.
  - qkv:   q,k in [feat, tok] layout; v in [tok, feat] layout.
  - logits per head, K=64, two heads run concurrently via PE row groups.
  - talking-heads mixing runs as 120x120 block-diagonal matmuls in a packed
    layout [(h-major: p = 10h + n_i), m] over blocks of 10 query rows.
    The partition interleave that builds this layout round-trips through a
    DRAM scratch laid out [h][ni][block][m_pad]: 12 block writes per qtile
    (1168B-aligned runs) + ONE fully-contiguous packed read per qtile
    ([120 partitions x 14KB runs]), all on otherwise-idle DMA queues.
  - softmax without max-subtraction (logits are small); exp on ScalarE with
    accum_out producing the row sums.  The 1/rowsum normalization is folded
    into the tiny 120x120 post-mix matrix (scale its rows per-partition)
    instead of rescaling the big [120, 577] exp tensor.
  - post-mix is fused with the transpose AV needs: E-tile is the stationary
    operand, the sinv-scaled block-diag Wpost^T the moving one, giving
    P'^T[m, (10g+n)] in PSUM directly.
  - AV consumes P'^T with a strided free AP per head; head pairs run
    concurrently via PE column groups. Output lands in [feat, tok] layout,
    which feeds the final projection without any transpose.
  - emission is block-interleaved (postmix/AV/proj of qtile j alongside
    logits of qtile j+2 and premix of j+1) over one shared 4-deep PSUM pool
    so the PE always has queued work and the HAM clock gate stays warm.
"""

import numpy as np
import ml_dtypes

import concourse.bass as bass
import concourse.mybir as mybir
from concourse import bacc
from concourse.tile import TileContext
from concourse.bass_utils import run_bass_kernel_spmd

BF16 = ml_dtypes.bfloat16

B, N, C, H = 16, 577, 768, 12
D = C // H                 # 64
NCORES = 8
BPC = B // NCORES          # batches per core = 2
NPAD = 600                 # padded query-token count (5 qtiles of 120)
QT = 5                     # query tiles per batch
NJ = BPC * QT              # total qtile jobs per core = 10
QTW = 120                  # rows per query tile
NI = 10                    # query rows per packed block
BPQ = QTW // NI            # blocks per qtile = 12
FT = C // 128              # feature tiles = 6
MT = [128, 128, 128, 128, 65]   # key-token tiles (sum 577)
MOF = [0, 128, 256, 384, 512]
MP = 584                   # padded key count for the pack scratch (16B runs)

_NC_CACHE = {}


def _build_nc():
    nc = bacc.Bacc("TRN2", target_bir_lowering=False)
    dt = mybir.dt

    xT = nc.dram_tensor("xT", [BPC, C, NPAD], dt.bfloat16, kind="ExternalInput")
    wqT = nc.dram_tensor("wqT", [C, C], dt.bfloat16, kind="ExternalInput")
    wkT = nc.dram_tensor("wkT", [C, C], dt.bfloat16, kind="ExternalInput")
    wvT = nc.dram_tensor("wvT", [C, C], dt.bfloat16, kind="ExternalInput")
    wpT = nc.dram_tensor("wpT", [C, C], dt.bfloat16, kind="ExternalInput")
    bdpre = nc.dram_tensor("bdpre", [QTW, QTW], dt.bfloat16, kind="ExternalInput")
    bdpostT = nc.dram_tensor("bdpostT", [QTW, QTW], dt.bfloat16, kind="ExternalInput")
    bias = nc.dram_tensor("bias", [C], dt.float32, kind="ExternalInput")
    y = nc.dram_tensor("y", [BPC, N, C], dt.float32, kind="ExternalOutput")
    # packed-logits scratch: [batch][qtile][h][ni][block][m] so that the
    # packed read is one [120 part x 14KB contiguous] DMA per qtile.
    pk = nc.dram_tensor("pk", [BPC, QT, H, NI, BPQ, MP], dt.bfloat16,
                        kind="Internal")

    with TileContext(nc) as tc:
        with (
            tc.tile_pool(name="consts", bufs=1) as consts,
            tc.tile_pool(name="io1", bufs=1) as io1,        # xT, q, k, o
            tc.tile_pool(name="vpool", bufs=2) as vpool,    # v (alive across batch overlap)
            tc.tile_pool(name="lnatp", bufs=2) as lnatp,
            tc.tile_pool(name="lpkp", bufs=2) as lpkp,
            tc.tile_pool(name="ep", bufs=2) as ep,
            tc.tile_pool(name="ptp", bufs=1) as ptp,
            tc.tile_pool(name="stage", bufs=2) as stage,
            tc.tile_pool(name="outp", bufs=2) as outp,
            tc.tile_pool(name="psp", bufs=2, space="PSUM") as psp,
            tc.tile_pool(name="psq", bufs=4, space="PSUM") as psq,
        ):
            # ---- constants (xT of batch 0 + q/k weights first) ----
            wq_sb = consts.tile([128, FT, C], dt.bfloat16, tag="wq")
            wk_sb = consts.tile([128, FT, C], dt.bfloat16, tag="wk")
            wv_sb = consts.tile([128, FT, C], dt.bfloat16, tag="wv")
            wp_sb = consts.tile([128, FT, C], dt.bfloat16, tag="wp")
            for w_sb, w_dr, eng in ((wq_sb, wqT, nc.scalar), (wk_sb, wkT, nc.gpsimd),
                                    (wv_sb, wvT, nc.scalar), (wp_sb, wpT, nc.gpsimd)):
                eng.dma_start(out=w_sb[:], in_=w_dr.rearrange("(t p) f -> p t f", p=128))
            bdpre_sb = consts.tile([QTW, QTW], dt.bfloat16, tag="bdpre")
            nc.scalar.dma_start(out=bdpre_sb[:], in_=bdpre[:])
            bdpostT_sb = consts.tile([QTW, QTW], dt.bfloat16, tag="bdpostT")
            nc.gpsimd.dma_start(out=bdpostT_sb[:], in_=bdpostT[:])
            bias_sb = consts.tile([128, C], dt.float32, tag="bias")
            nc.scalar.dma_start(
                out=bias_sb[:],
                in_=bass.AP(tensor=bias[:].tensor, offset=0, ap=[[0, 128], [1, C]]),
            )

            # per-batch SBUF state, filled by emit_load/emit_qkv
            st = [dict() for _ in range(BPC)]

            def emit_load(bi):
                xT_sb = io1.tile([128, FT, NPAD], dt.bfloat16, tag="xT")
                nc.sync.dma_start(
                    out=xT_sb[:], in_=xT[bi].rearrange("(t p) n -> p t n", p=128)
                )
                st[bi]["xT"] = xT_sb

            def qkv_units(bi, which="all"):
                """thunks for qkv stage; which in ('qk', 'v', 'all').
                q/k units must precede logits; v is only needed by AV."""
                xT_sb = st[bi]["xT"]
                if "q" not in st[bi]:
                    q_sb = io1.tile([128, FT, NPAD], dt.bfloat16, tag="q")
                    k_sb = io1.tile([128, FT, MP], dt.bfloat16, tag="k")
                    v_sb = vpool.tile([128, len(MT), C], dt.bfloat16, tag="v")
                    st[bi].update(q=q_sb, k=k_sb, v=v_sb)
                q_sb, k_sb, v_sb = st[bi]["q"], st[bi]["k"], st[bi]["v"]
                units = []
                uidx = [0]

                def pad_unit():
                    # zero k's pad columns so padded logits are exactly 0
                    nc.vector.memset(k_sb[:, :, N:MP], 0.0)

                def qk_unit(ft, which):
                    def emit():
                        dst, w_sb, ntok = (
                            (q_sb, wq_sb, NPAD) if which == 0 else (k_sb, wk_sb, N)
                        )
                        for lo, hi in ((0, 512), (512, ntok)):
                            ps = psq.tile([128, 512], dt.float32, tag="psq")
                            for kc in range(FT):
                                nc.tensor.matmul(
                                    out=ps[:, 0:hi - lo],
                                    lhsT=w_sb[:, kc, ft * 128:(ft + 1) * 128],
                                    rhs=xT_sb[:, kc, lo:hi],
                                    start=(kc == 0), stop=(kc == FT - 1),
                                )
                            if uidx[0] % 2 == 0:
                                nc.vector.tensor_copy(out=dst[:, ft, lo:hi],
                                                      in_=ps[:, 0:hi - lo])
                            else:
                                nc.scalar.copy(out=dst[:, ft, lo:hi],
                                               in_=ps[:, 0:hi - lo])
                            uidx[0] += 1
                    return emit

                def v_unit(mt):
                    def emit():
                        mw = MT[mt]
                        for lo, hi in ((0, 512), (512, C)):
                            ps = psq.tile([128, 512], dt.float32, tag="psq")
                            for kc in range(FT):
                                nc.tensor.matmul(
                                    out=ps[0:mw, 0:hi - lo],
                                    lhsT=xT_sb[:, kc, MOF[mt]:MOF[mt] + mw],
                                    rhs=wv_sb[:, kc, lo:hi],
                                    start=(kc == 0), stop=(kc == FT - 1),
                                )
                            if uidx[0] % 2 == 0:
                                nc.vector.tensor_copy(out=v_sb[0:mw, mt, lo:hi],
                                                      in_=ps[0:mw, 0:hi - lo])
                            else:
                                nc.scalar.copy(out=v_sb[0:mw, mt, lo:hi],
                                               in_=ps[0:mw, 0:hi - lo])
                            uidx[0] += 1
                    return emit

                if which in ("qk", "all"):
                    units.append(pad_unit)
                    for ft in range(FT):
                        units.append(qk_unit(ft, 0))
                        units.append(qk_unit(ft, 1))
                if which in ("v", "all"):
                    for mt in range(len(MT)):
                        units.append(v_unit(mt))
                return units

            # ---- per-job state (j = bi*QT + qt) ----
            lnats = {}
            lpks = {}
            es = {}
            ss = {}
            bdscs = {}
            pts = {}

            def L_units(j):
                """logits of job j: 6 head-pair units writing l_nat."""
                bi, qt = divmod(j, QT)
                q0 = qt * QTW
                q_sb, k_sb = st[bi]["q"], st[bi]["k"]
                l_nat = lnatp.tile([QTW, H, MP], dt.bfloat16, tag="lnat")
                lnats[j] = l_nat
                units = []

                def hp_unit(hp):
                    def emit():
                        ps0 = psp.tile([QTW, MP], dt.float32, tag="ps")
                        ps1 = psp.tile([QTW, MP], dt.float32, tag="ps")
                        for sub, ps in ((0, ps0), (1, ps1)):
                            pbase = 64 * sub
                            for lo, hi in ((0, 512), (512, MP)):
                                nc.tensor.matmul(
                                    out=ps[:, lo:hi],
                                    lhsT=q_sb[pbase:pbase + 64, hp, q0:q0 + QTW],
                                    rhs=k_sb[pbase:pbase + 64, hp, lo:hi],
                                )
                        for sub, ps in ((0, ps0), (1, ps1)):
                            idx = 2 * hp + sub
                            dst = l_nat[:, idx, :]
                            if idx in (3, 7, 11):
                                nc.scalar.copy(out=dst, in_=ps[:])
                            else:
                                nc.vector.tensor_copy(out=dst, in_=ps[:])
                            # per-head pack write, gated only on this evac:
                            # overlaps the rest of the L stage.
                            weng = nc.gpsimd if idx % 2 == 0 else nc.sync
                            weng.dma_start(
                                out=pk[bi, qt, idx].rearrange("ni b m -> b ni m"),
                                in_=l_nat[:, idx, :],
                            )
                    return emit

                for hp in range(H // 2):
                    units.append(hp_unit(hp))
                return units

            def emit_read(j):
                bi, qt = divmod(j, QT)
                l_pk = lpkp.tile([QTW, BPQ, MP], dt.bfloat16, tag="lpk")
                lpks[j] = l_pk
                hb = BPQ // 2
                # two HWDGE rings in parallel; the ACT-ring issue is emitted
                # late enough (after the pack writes land) not to stall ACT.
                for b0, eng in ((0, nc.sync), (hb, nc.scalar)):
                    eng.dma_start(
                        out=l_pk[:, b0:b0 + hb, :].rearrange("p b m -> p (b m)"),
                        in_=pk[bi, qt, :, :, b0:b0 + hb, :].rearrange(
                            "h ni b m -> (h ni) (b m)"),
                    )

            def P_units(j):
                """premix + exp of job j (needs lpks[j]); then recip + bdscale."""
                l_pk = lpks[j]
                e_sb = ep.tile([QTW, BPQ, MP], dt.bfloat16, tag="e")
                s_sb = stage.tile([QTW, BPQ], dt.float32, tag="s")
                es[j], ss[j] = e_sb, s_sb
                units = []

                def pm_unit(b):
                    def emit():
                        ps = psp.tile([QTW, MP], dt.float32, tag="ps")
                        for lo, hi in ((0, 512), (512, N)):
                            nc.tensor.matmul(
                                out=ps[:, lo:hi], lhsT=bdpre_sb[:], rhs=l_pk[:, b, lo:hi]
                            )
                        nc.scalar.activation(
                            out=e_sb[:, b, 0:N], in_=ps[:, 0:N],
                            func=mybir.ActivationFunctionType.Exp,
                            accum_out=s_sb[:, b:b + 1],
                        )
                    return emit

                sinv = stage.tile([QTW, BPQ], dt.float32, tag="sinv")
                bdsc = stage.tile([QTW, BPQ, QTW], dt.bfloat16, tag="bdsc")
                bdscs[j] = bdsc
                hb = BPQ // 2

                def scale_unit(half):
                    def emit():
                        # per-half so the first postmix groups don't wait for
                        # the last exps of this job
                        lo = half * hb
                        nc.vector.reciprocal(
                            out=sinv[:, lo:lo + hb], in_=s_sb[:, lo:lo + hb])
                        for b in range(lo, lo + hb):
                            nc.vector.tensor_scalar_mul(
                                bdsc[:, b, :], bdpostT_sb[:], sinv[:, b:b + 1]
                            )
                    return emit

                for b in range(BPQ):
                    units.append(pm_unit(b))
                    if b == hb - 1:
                        units.append(scale_unit(0))
                units.append(scale_unit(1))
                return units

            def Q1_units(j):
                """postmix (fused with transpose): 15 groups of 4 blocks."""
                e_sb = es[j]
                pt_sb = ptp.tile([128, len(MT), BPQ, QTW], dt.bfloat16, tag="pt")
                pts[j] = pt_sb
                bdsc = bdscs[j]
                units = []

                def grp_unit(mt, bg, idx):
                    def emit():
                        mw = MT[mt]
                        ps = psq.tile([128, 4 * QTW], dt.float32, tag="psq")
                        for sl in range(4):
                            b = 4 * bg + sl
                            nc.tensor.matmul(
                                out=ps[0:mw, sl * QTW:(sl + 1) * QTW],
                                lhsT=e_sb[:, b, MOF[mt]:MOF[mt] + mw],
                                rhs=bdsc[:, b, :],
                            )
                        dst = pt_sb[0:mw, mt, 4 * bg:4 * (bg + 1), :]
                        if idx % 3 == 0:
                            nc.scalar.copy(out=dst, in_=ps[0:mw, 0:4 * QTW])
                        else:
                            nc.vector.tensor_copy(out=dst, in_=ps[0:mw, 0:4 * QTW])
                    return emit

                # bg-major: the first groups only need the first half of bdsc
                idx = 0
                for bg in range(BPQ // 4):
                    for mt in range(len(MT)):
                        units.append(grp_unit(mt, bg, idx))
                        idx += 1
                return units

            def get_o(bi):
                # allocated lazily: first writer is AV of (bi, qt0), which on
                # the PE stream comes after proj of (bi-1, qt4) — the last
                # reader of the previous batch's o — keeping bufs=1 feasible.
                if "o" not in st[bi]:
                    o_sb = io1.tile([128, FT, NPAD], dt.bfloat16, tag="o")
                    st[bi]["o"] = o_sb
                return st[bi]["o"]

            def Q2_units(j):
                """AV: head pairs via PE column groups, into o[feat, tok]."""
                bi, qt = divmod(j, QT)
                q0 = qt * QTW
                v_sb, o_sb = st[bi]["v"], get_o(bi)
                pt_sb = pts[j]
                units = []

                def gpp_unit(a):
                    def emit():
                        # two head-pairs (4 heads) share one 1-bank PSUM tile
                        ps = psq.tile([128, 2 * QTW], dt.float32, tag="psq")
                        for half in range(2):
                            gp = 2 * a + half
                            for sub in range(2):
                                g = 2 * gp + sub
                                for mt in range(len(MT)):
                                    mw = MT[mt]
                                    nc.tensor.matmul(
                                        out=ps[64 * sub:64 * (sub + 1),
                                               half * QTW:(half + 1) * QTW],
                                        lhsT=v_sb[0:mw, mt, 64 * g:64 * (g + 1)],
                                        rhs=pt_sb[0:mw, mt, :, NI * g:NI * (g + 1)],
                                        start=(mt == 0), stop=(mt == len(MT) - 1),
                                        skip_group_check=True,
                                    )
                        nc.vector.tensor_copy(
                            out=o_sb[:, 2 * a:2 * a + 2, q0:q0 + QTW],
                            in_=ps[:].rearrange("p (h n) -> p h n", h=2),
                        )
                    return emit

                for a in range(H // 4):
                    units.append(gpp_unit(a))
                return units

            def Q3_unit(j):
                """output projection + bias + y write for job j."""
                bi, qt = divmod(j, QT)
                q0 = qt * QTW
                o_sb = get_o(bi)
                out_sb = outp.tile([QTW, C], dt.float32, tag="out")
                for lo, hi in ((0, 512), (512, C)):
                    ps = psq.tile([QTW, 512], dt.float32, tag="psq")
                    for kc in range(FT):
                        nc.tensor.matmul(
                            out=ps[:, 0:hi - lo],
                            lhsT=o_sb[:, kc, q0:q0 + QTW],
                            rhs=wp_sb[:, kc, lo:hi],
                            start=(kc == 0), stop=(kc == FT - 1),
                        )
                    nc.vector.tensor_tensor(
                        out=out_sb[:, lo:hi], in0=ps[:, 0:hi - lo],
                        in1=bias_sb[0:QTW, lo:hi], op=mybir.AluOpType.add,
                    )
                rows = min(N - q0, QTW)
                nc.gpsimd.dma_start(out=y[bi, q0:q0 + rows, :], in_=out_sb[0:rows, :])

            # ================= global schedule =================
            # prologue: q/k first, logits of job 0 ASAP (its pack round-trip
            # is on the critical path to the first premix); v afterwards.
            emit_load(0)
            for u in qkv_units(0, "qk"):
                u()
            for u in L_units(0):
                u()
            emit_read(0)
            for u in qkv_units(0, "v"):
                u()
            for u in L_units(1):
                u()
            emit_read(1)
            for u in P_units(0):
                u()

            for j in range(NJ):
                # L(j+2) emitted densely at block start so its evacs + pack
                # round-trip complete ~1.5 blocks before premix(j+2) needs it.
                if j == 1:
                    emit_load(1)
                if j + 2 < NJ:
                    for u in L_units(j + 2):
                        u()
                if j == 2:
                    # batch-1 qkv AFTER L(4): L(4) is the last reader of
                    # batch-0's q/k, whose slots (bufs=1) batch 1 reuses.
                    for u in qkv_units(1, "all"):
                        u()
                for u in Q1_units(j):
                    u()
                if j + 2 < NJ:
                    # after Q1 so the ACT-ring dma_start never blocks evacs
                    emit_read(j + 2)
                for u in Q2_units(j):
                    u()
                Q3_unit(j)
                if j + 1 < NJ:
                    for u in P_units(j + 1):
                        u()
                # release dead per-job references
                for dd in (lnats, lpks, es, ss, bdscs, pts):
                    dd.pop(j, None)
    nc.compile()
    return nc


def _host_prep(x, Wqkv, Wproj, bproj, Wpre, Wpost):
    scale = D ** -0.5
    Wq = (Wqkv[0:C] * scale).T        # [C, C] lhsT for q (scale folded)
    Wk = Wqkv[C:2 * C].T
    Wv = Wqkv[2 * C:3 * C].T
    Wp = Wproj.T
    # h-major packed-block mixing matrices (p = 10*h + n_i)
    eye = np.eye(NI, dtype=np.float32)
    # bdpre[(10h+ni), (10g+nj)] = Wpre[g, h] * (ni == nj)
    bdpre = np.einsum("gh,ij->higj", Wpre.astype(np.float32), eye).reshape(QTW, QTW)
    # bdpostT[(10g+ni), (10g'+nj)] = Wpost[g', g] * (ni == nj)
    bdpostT = np.einsum("pg,ij->gipj", Wpost.astype(np.float32), eye).reshape(QTW, QTW)

    xT = np.zeros((B, C, NPAD), dtype=BF16)
    xT[:, :, 0:N] = np.ascontiguousarray(x.transpose(0, 2, 1)).astype(BF16)
    return {
        "xT": xT,
        "wqT": np.ascontiguousarray(Wq).astype(BF16),
        "wkT": np.ascontiguousarray(Wk).astype(BF16),
        "wvT": np.ascontiguousarray(Wv).astype(BF16),
        "wpT": np.ascontiguousarray(Wp).astype(BF16),
        "bdpre": bdpre.astype(BF16),
        "bdpostT": bdpostT.astype(BF16),
        "bias": bproj.astype(np.float32),
    }


def kernel(x, Wqkv, Wproj, bproj, Wpre, Wpost):
    x = np.asarray(x, dtype=np.float32)
    Wqkv = np.asarray(Wqkv, dtype=np.float32)
    Wproj = np.asarray(Wproj, dtype=np.float32)
    bproj = np.asarray(bproj, dtype=np.float32)
    Wpre = np.asarray(Wpre, dtype=np.float32)
    Wpost = np.asarray(Wpost, dtype=np.float32)

    host = _host_prep(x, Wqkv, Wproj, bproj, Wpre, Wpost)
    if "nc" not in _NC_CACHE:
        _NC_CACHE["nc"] = _build_nc()
    nc = _NC_CACHE["nc"]

    shared = {k: host[k] for k in
              ("wqT", "wkT", "wvT", "wpT", "bdpre", "bdpostT", "bias")}
    in_maps = []
    for core in range(NCORES):
        m = dict(shared)
        m["xT"] = host["xT"][core * BPC:(core + 1) * BPC]
        in_maps.append(m)

    res = run_bass_kernel_spmd(nc, in_maps, core_ids=list(range(NCORES)))
    out = np.concatenate([np.asarray(r["y"]) for r in res.results], axis=0)
    return out.astype(np.float32)


# revision 25
# speedup vs baseline: 1.2094x; 1.1576x over previous
"""Talking-heads attention (ViT-B/16-ish shapes) on 8 Trainium2 NeuronCores.

Problem: B=16, N=577, C=768, H=12 heads, d=64.
  qkv = x @ Wqkv.T ; logits = q k^T * scale ; pre-softmax head mix (Wpre);
  softmax ; post-softmax head mix (Wpost) ; out = (attn @ v) @ Wproj.T + b.

Distribution: pure data-parallel over batch, 2 batches per core, no
collectives.

Per-core design (all matmuls bf16 inputs, fp32 PSUM accumulation):
  - host pre-transposes x to [C, N] and pre-casts/packs all weights.
  - qkv:   q,k in [feat, tok] layout; v in [tok, feat] layout.
  - logits per head, K=64, two heads run concurrently via PE row groups.
  - talking-heads mixing runs as 120x120 block-diagonal matmuls in a packed
    layout [(h-major: p = 10h + n_i), m] over blocks of 10 query rows.
    The partition interleave that builds this layout round-trips through a
    DRAM scratch laid out [h][ni][block][m_pad]: 12 block writes per qtile
    (1168B-aligned runs) + ONE fully-contiguous packed read per qtile
    ([120 partitions x 14KB runs]), all on otherwise-idle DMA queues.
  - softmax without max-subtraction (logits are small); exp on ScalarE with
    accum_out producing the row sums.  The 1/rowsum normalization is folded
    into the tiny 120x120 post-mix matrix (scale its rows per-partition)
    instead of rescaling the big [120, 577] exp tensor.
  - post-mix is fused with the transpose AV needs: E-tile is the stationary
    operand, the sinv-scaled block-diag Wpost^T the moving one, giving
    P'^T[m, (10g+n)] in PSUM directly.
  - AV consumes P'^T with a strided free AP per head; head pairs run
    concurrently via PE column groups. Output lands in [feat, tok] layout,
    which feeds the final projection without any transpose.
  - emission is block-interleaved (postmix/AV/proj of qtile j alongside
    logits of qtile j+2 and premix of j+1) over one shared 4-deep PSUM pool
    so the PE always has queued work and the HAM clock gate stays warm.
"""

import numpy as np
import ml_dtypes

import concourse.bass as bass
import concourse.mybir as mybir
from concourse import bacc
from concourse.tile import TileContext
from concourse.bass_utils import run_bass_kernel_spmd

BF16 = ml_dtypes.bfloat16

B, N, C, H = 16, 577, 768, 12
D = C // H                 # 64
NCORES = 8
BPC = B // NCORES          # batches per core = 2
NPAD = 600                 # padded query-token count (5 qtiles of 120)
QT = 5                     # query tiles per batch
NJ = BPC * QT              # total qtile jobs per core = 10
QTW = 120                  # rows per query tile
NI = 10                    # query rows per packed block
BPQ = QTW // NI            # blocks per qtile = 12
FT = C // 128              # feature tiles = 6
MT = [128, 128, 128, 128, 65]   # key-token tiles (sum 577)
MOF = [0, 128, 256, 384, 512]
MP = 584                   # padded key count for the pack scratch (16B runs)

_NC_CACHE = {}


def _build_nc():
    nc = bacc.Bacc("TRN2", target_bir_lowering=False)
    dt = mybir.dt

    xT = nc.dram_tensor("xT", [BPC, C, NPAD], dt.bfloat16, kind="ExternalInput")
    wqT = nc.dram_tensor("wqT", [C, C], dt.bfloat16, kind="ExternalInput")
    wkT = nc.dram_tensor("wkT", [C, C], dt.bfloat16, kind="ExternalInput")
    wvT = nc.dram_tensor("wvT", [C, C], dt.bfloat16, kind="ExternalInput")
    wpT = nc.dram_tensor("wpT", [C, C], dt.bfloat16, kind="ExternalInput")
    bdpre = nc.dram_tensor("bdpre", [QTW, QTW], dt.bfloat16, kind="ExternalInput")
    bdpostT = nc.dram_tensor("bdpostT", [QTW, QTW], dt.bfloat16, kind="ExternalInput")
    bias = nc.dram_tensor("bias", [C], dt.float32, kind="ExternalInput")
    y = nc.dram_tensor("y", [BPC, N, C], dt.float32, kind="ExternalOutput")
    # packed-logits scratch: [batch][qtile][h][ni][block][m] so that the
    # packed read is one [120 part x 14KB contiguous] DMA per qtile.
    pk = nc.dram_tensor("pk", [BPC, QT, H, NI, BPQ, MP], dt.bfloat16,
                        kind="Internal")

    with TileContext(nc) as tc:
        with (
            tc.tile_pool(name="consts", bufs=1) as consts,
            tc.tile_pool(name="io1", bufs=1) as io1,        # xT, q, k, o
            tc.tile_pool(name="vpool", bufs=2) as vpool,    # v (alive across batch overlap)
            tc.tile_pool(name="lnatp", bufs=2) as lnatp,
            tc.tile_pool(name="lpkp", bufs=2) as lpkp,
            tc.tile_pool(name="ep", bufs=2) as ep,
            tc.tile_pool(name="ptp", bufs=1) as ptp,
            tc.tile_pool(name="stage", bufs=2) as stage,
            tc.tile_pool(name="outp", bufs=2) as outp,
            tc.tile_pool(name="psp", bufs=2, space="PSUM") as psp,
            tc.tile_pool(name="psq", bufs=4, space="PSUM") as psq,
        ):
            # ---- constants (xT of batch 0 + q/k weights first) ----
            wq_sb = consts.tile([128, FT, C], dt.bfloat16, tag="wq")
            wk_sb = consts.tile([128, FT, C], dt.bfloat16, tag="wk")
            wv_sb = consts.tile([128, FT, C], dt.bfloat16, tag="wv")
            wp_sb = consts.tile([128, FT, C], dt.bfloat16, tag="wp")
            for w_sb, w_dr, eng in ((wq_sb, wqT, nc.scalar), (wk_sb, wkT, nc.gpsimd),
                                    (wv_sb, wvT, nc.scalar), (wp_sb, wpT, nc.gpsimd)):
                eng.dma_start(out=w_sb[:], in_=w_dr.rearrange("(t p) f -> p t f", p=128))
            bdpre_sb = consts.tile([QTW, QTW], dt.bfloat16, tag="bdpre")
            nc.scalar.dma_start(out=bdpre_sb[:], in_=bdpre[:])
            bdpostT_sb = consts.tile([QTW, QTW], dt.bfloat16, tag="bdpostT")
            nc.gpsimd.dma_start(out=bdpostT_sb[:], in_=bdpostT[:])
            bias_sb = consts.tile([128, C], dt.float32, tag="bias")
            nc.scalar.dma_start(
                out=bias_sb[:],
                in_=bass.AP(tensor=bias[:].tensor, offset=0, ap=[[0, 128], [1, C]]),
            )

            # per-batch SBUF state, filled by emit_load/emit_qkv
            st = [dict() for _ in range(BPC)]

            def emit_load(bi):
                xT_sb = io1.tile([128, FT, NPAD], dt.bfloat16, tag="xT")
                nc.sync.dma_start(
                    out=xT_sb[:], in_=xT[bi].rearrange("(t p) n -> p t n", p=128)
                )
                st[bi]["xT"] = xT_sb

            def qkv_units(bi, which="all"):
                """thunks for qkv stage; which in ('qk', 'v', 'all').
                q/k units must precede logits; v is only needed by AV."""
                xT_sb = st[bi]["xT"]
                if "q" not in st[bi]:
                    q_sb = io1.tile([128, FT, NPAD], dt.bfloat16, tag="q")
                    k_sb = io1.tile([128, FT, MP], dt.bfloat16, tag="k")
                    v_sb = vpool.tile([128, len(MT), C], dt.bfloat16, tag="v")
                    st[bi].update(q=q_sb, k=k_sb, v=v_sb)
                q_sb, k_sb, v_sb = st[bi]["q"], st[bi]["k"], st[bi]["v"]
                units = []
                uidx = [0]

                def pad_unit():
                    # zero k's pad columns so padded logits are exactly 0
                    nc.vector.memset(k_sb[:, :, N:MP], 0.0)

                def qk_unit(ft, which):
                    def emit():
                        dst, w_sb, ntok = (
                            (q_sb, wq_sb, NPAD) if which == 0 else (k_sb, wk_sb, N)
                        )
                        ps = psp.tile([128, NPAD], dt.float32, tag="ps")
                        for kc in range(FT):
                            for lo, hi in ((0, 512), (512, ntok)):
                                nc.tensor.matmul(
                                    out=ps[:, lo:hi],
                                    lhsT=w_sb[:, kc, ft * 128:(ft + 1) * 128],
                                    rhs=xT_sb[:, kc, lo:hi],
                                    start=(kc == 0), stop=(kc == FT - 1),
                                )
                        if uidx[0] % 2 == 0:
                            nc.vector.tensor_copy(out=dst[:, ft, 0:ntok], in_=ps[:, 0:ntok])
                        else:
                            nc.scalar.copy(out=dst[:, ft, 0:ntok], in_=ps[:, 0:ntok])
                        uidx[0] += 1
                    return emit

                def v_unit(mt):
                    def emit():
                        ps = psp.tile([128, C], dt.float32, tag="ps")
                        mw = MT[mt]
                        for kc in range(FT):
                            for lo, hi in ((0, 512), (512, C)):
                                nc.tensor.matmul(
                                    out=ps[0:mw, lo:hi],
                                    lhsT=xT_sb[:, kc, MOF[mt]:MOF[mt] + mw],
                                    rhs=wv_sb[:, kc, lo:hi],
                                    start=(kc == 0), stop=(kc == FT - 1),
                                )
                        if uidx[0] % 2 == 0:
                            nc.vector.tensor_copy(out=v_sb[0:mw, mt, :], in_=ps[0:mw, 0:C])
                        else:
                            nc.scalar.copy(out=v_sb[0:mw, mt, :], in_=ps[0:mw, 0:C])
                        uidx[0] += 1
                    return emit

                if which in ("qk", "all"):
                    units.append(pad_unit)
                    for ft in range(FT):
                        units.append(qk_unit(ft, 0))
                        units.append(qk_unit(ft, 1))
                if which in ("v", "all"):
                    for mt in range(len(MT)):
                        units.append(v_unit(mt))
                return units

            # ---- per-job state (j = bi*QT + qt) ----
            lnats = {}
            lpks = {}
            es = {}
            ss = {}
            bdscs = {}
            pts = {}

            def L_units(j):
                """logits of job j: 6 head-pair units writing l_nat."""
                bi, qt = divmod(j, QT)
                q0 = qt * QTW
                q_sb, k_sb = st[bi]["q"], st[bi]["k"]
                l_nat = lnatp.tile([QTW, H, MP], dt.bfloat16, tag="lnat")
                lnats[j] = l_nat
                units = []

                def hp_unit(hp):
                    def emit():
                        ps0 = psp.tile([QTW, MP], dt.float32, tag="ps")
                        ps1 = psp.tile([QTW, MP], dt.float32, tag="ps")
                        for sub, ps in ((0, ps0), (1, ps1)):
                            pbase = 64 * sub
                            for lo, hi in ((0, 512), (512, MP)):
                                nc.tensor.matmul(
                                    out=ps[:, lo:hi],
                                    lhsT=q_sb[pbase:pbase + 64, hp, q0:q0 + QTW],
                                    rhs=k_sb[pbase:pbase + 64, hp, lo:hi],
                                )
                        for sub, ps in ((0, ps0), (1, ps1)):
                            idx = 2 * hp + sub
                            dst = l_nat[:, idx, :]
                            if idx in (3, 7, 11):
                                nc.scalar.copy(out=dst, in_=ps[:])
                            else:
                                nc.vector.tensor_copy(out=dst, in_=ps[:])
                            # per-head pack write, gated only on this evac:
                            # overlaps the rest of the L stage.
                            nc.gpsimd.dma_start(
                                out=pk[bi, qt, idx].rearrange("ni b m -> b ni m"),
                                in_=l_nat[:, idx, :],
                            )
                    return emit

                for hp in range(H // 2):
                    units.append(hp_unit(hp))
                return units

            def emit_read(j):
                bi, qt = divmod(j, QT)
                l_pk = lpkp.tile([QTW, BPQ, MP], dt.bfloat16, tag="lpk")
                lpks[j] = l_pk
                hb = BPQ // 2
                for b0 in (0, hb):
                    nc.sync.dma_start(
                        out=l_pk[:, b0:b0 + hb, :].rearrange("p b m -> p (b m)"),
                        in_=pk[bi, qt, :, :, b0:b0 + hb, :].rearrange(
                            "h ni b m -> (h ni) (b m)"),
                    )

            def P_units(j):
                """premix + exp of job j (needs lpks[j]); then recip + bdscale."""
                l_pk = lpks[j]
                e_sb = ep.tile([QTW, BPQ, MP], dt.bfloat16, tag="e")
                s_sb = stage.tile([QTW, BPQ], dt.float32, tag="s")
                es[j], ss[j] = e_sb, s_sb
                units = []

                def pm_unit(b):
                    def emit():
                        ps = psp.tile([QTW, MP], dt.float32, tag="ps")
                        for lo, hi in ((0, 512), (512, N)):
                            nc.tensor.matmul(
                                out=ps[:, lo:hi], lhsT=bdpre_sb[:], rhs=l_pk[:, b, lo:hi]
                            )
                        nc.scalar.activation(
                            out=e_sb[:, b, 0:N], in_=ps[:, 0:N],
                            func=mybir.ActivationFunctionType.Exp,
                            accum_out=s_sb[:, b:b + 1],
                        )
                    return emit

                sinv = stage.tile([QTW, BPQ], dt.float32, tag="sinv")
                bdsc = stage.tile([QTW, BPQ, QTW], dt.bfloat16, tag="bdsc")
                bdscs[j] = bdsc
                hb = BPQ // 2

                def scale_unit(half):
                    def emit():
                        # per-half so the first postmix groups don't wait for
                        # the last exps of this job
                        lo = half * hb
                        nc.vector.reciprocal(
                            out=sinv[:, lo:lo + hb], in_=s_sb[:, lo:lo + hb])
                        for b in range(lo, lo + hb):
                            nc.vector.tensor_scalar_mul(
                                bdsc[:, b, :], bdpostT_sb[:], sinv[:, b:b + 1]
                            )
                    return emit

                for b in range(BPQ):
                    units.append(pm_unit(b))
                    if b == hb - 1:
                        units.append(scale_unit(0))
                units.append(scale_unit(1))
                return units

            def Q1_units(j):
                """postmix (fused with transpose): 15 groups of 4 blocks."""
                e_sb = es[j]
                pt_sb = ptp.tile([128, len(MT), BPQ, QTW], dt.bfloat16, tag="pt")
                pts[j] = pt_sb
                bdsc = bdscs[j]
                units = []

                def grp_unit(mt, bg, idx):
                    def emit():
                        mw = MT[mt]
                        ps = psq.tile([128, 4 * QTW], dt.float32, tag="psq")
                        for sl in range(4):
                            b = 4 * bg + sl
                            nc.tensor.matmul(
                                out=ps[0:mw, sl * QTW:(sl + 1) * QTW],
                                lhsT=e_sb[:, b, MOF[mt]:MOF[mt] + mw],
                                rhs=bdsc[:, b, :],
                            )
                        dst = pt_sb[0:mw, mt, 4 * bg:4 * (bg + 1), :]
                        if idx % 3 == 0:
                            nc.scalar.copy(out=dst, in_=ps[0:mw, 0:4 * QTW])
                        else:
                            nc.vector.tensor_copy(out=dst, in_=ps[0:mw, 0:4 * QTW])
                    return emit

                # bg-major: the first groups only need the first half of bdsc
                idx = 0
                for bg in range(BPQ // 4):
                    for mt in range(len(MT)):
                        units.append(grp_unit(mt, bg, idx))
                        idx += 1
                return units

            def get_o(bi):
                # allocated lazily: first writer is AV of (bi, qt0), which on
                # the PE stream comes after proj of (bi-1, qt4) — the last
                # reader of the previous batch's o — keeping bufs=1 feasible.
                if "o" not in st[bi]:
                    o_sb = io1.tile([128, FT, NPAD], dt.bfloat16, tag="o")
                    st[bi]["o"] = o_sb
                return st[bi]["o"]

            def Q2_units(j):
                """AV: head pairs via PE column groups, into o[feat, tok]."""
                bi, qt = divmod(j, QT)
                q0 = qt * QTW
                v_sb, o_sb = st[bi]["v"], get_o(bi)
                pt_sb = pts[j]
                units = []

                def gp_unit(gp):
                    def emit():
                        ps = psq.tile([128, QTW], dt.float32, tag="psq")
                        for sub in range(2):
                            g = 2 * gp + sub
                            for mt in range(len(MT)):
                                mw = MT[mt]
                                nc.tensor.matmul(
                                    out=ps[64 * sub:64 * (sub + 1), :],
                                    lhsT=v_sb[0:mw, mt, 64 * g:64 * (g + 1)],
                                    rhs=pt_sb[0:mw, mt, :, NI * g:NI * (g + 1)],
                                    start=(mt == 0), stop=(mt == len(MT) - 1),
                                    skip_group_check=True,
                                )
                        nc.scalar.copy(out=o_sb[:, gp, q0:q0 + QTW], in_=ps[:])
                    return emit

                for gp in range(H // 2):
                    units.append(gp_unit(gp))
                return units

            def Q3_unit(j):
                """output projection + bias + y write for job j."""
                bi, qt = divmod(j, QT)
                q0 = qt * QTW
                o_sb = get_o(bi)
                ps = psp.tile([QTW, C], dt.float32, tag="ps")
                for kc in range(FT):
                    for lo, hi in ((0, 512), (512, C)):
                        nc.tensor.matmul(
                            out=ps[:, lo:hi],
                            lhsT=o_sb[:, kc, q0:q0 + QTW],
                            rhs=wp_sb[:, kc, lo:hi],
                            start=(kc == 0), stop=(kc == FT - 1),
                        )
                out_sb = outp.tile([QTW, C], dt.float32, tag="out")
                nc.vector.tensor_tensor(
                    out=out_sb[:], in0=ps[:], in1=bias_sb[0:QTW, :],
                    op=mybir.AluOpType.add,
                )
                rows = min(N - q0, QTW)
                nc.gpsimd.dma_start(out=y[bi, q0:q0 + rows, :], in_=out_sb[0:rows, :])

            # ================= global schedule =================
            # prologue: q/k first, logits of job 0 ASAP (its pack round-trip
            # is on the critical path to the first premix); v afterwards.
            emit_load(0)
            for u in qkv_units(0, "qk"):
                u()
            for u in L_units(0):
                u()
            emit_read(0)
            for u in qkv_units(0, "v"):
                u()
            for u in L_units(1):
                u()
            emit_read(1)
            for u in P_units(0):
                u()

            for j in range(NJ):
                # L(j+2) emitted densely at block start so its evacs + pack
                # round-trip complete ~1.5 blocks before premix(j+2) needs it.
                if j == 1:
                    emit_load(1)
                if j + 2 < NJ:
                    for u in L_units(j + 2):
                        u()
                    emit_read(j + 2)
                if j == 2:
                    # batch-1 qkv AFTER L(4): L(4) is the last reader of
                    # batch-0's q/k, whose slots (bufs=1) batch 1 reuses.
                    for u in qkv_units(1, "all"):
                        u()
                for u in Q1_units(j):
                    u()
                for u in Q2_units(j):
                    u()
                Q3_unit(j)
                if j + 1 < NJ:
                    for u in P_units(j + 1):
                        u()
                # release dead per-job references
                for dd in (lnats, lpks, es, ss, bdscs, pts):
                    dd.pop(j, None)
    nc.compile()
    return nc


def _host_prep(x, Wqkv, Wproj, bproj, Wpre, Wpost):
    scale = D ** -0.5
    Wq = (Wqkv[0:C] * scale).T        # [C, C] lhsT for q (scale folded)
    Wk = Wqkv[C:2 * C].T
    Wv = Wqkv[2 * C:3 * C].T
    Wp = Wproj.T
    # h-major packed-block mixing matrices (p = 10*h + n_i)
    eye = np.eye(NI, dtype=np.float32)
    # bdpre[(10h+ni), (10g+nj)] = Wpre[g, h] * (ni == nj)
    bdpre = np.einsum("gh,ij->higj", Wpre.astype(np.float32), eye).reshape(QTW, QTW)
    # bdpostT[(10g+ni), (10g'+nj)] = Wpost[g', g] * (ni == nj)
    bdpostT = np.einsum("pg,ij->gipj", Wpost.astype(np.float32), eye).reshape(QTW, QTW)

    xT = np.zeros((B, C, NPAD), dtype=BF16)
    xT[:, :, 0:N] = np.ascontiguousarray(x.transpose(0, 2, 1)).astype(BF16)
    return {
        "xT": xT,
        "wqT": np.ascontiguousarray(Wq).astype(BF16),
        "wkT": np.ascontiguousarray(Wk).astype(BF16),
        "wvT": np.ascontiguousarray(Wv).astype(BF16),
        "wpT": np.ascontiguousarray(Wp).astype(BF16),
        "bdpre": bdpre.astype(BF16),
        "bdpostT": bdpostT.astype(BF16),
        "bias": bproj.astype(np.float32),
    }


def kernel(x, Wqkv, Wproj, bproj, Wpre, Wpost):
    x = np.asarray(x, dtype=np.float32)
    Wqkv = np.asarray(Wqkv, dtype=np.float32)
    Wproj = np.asarray(Wproj, dtype=np.float32)
    bproj = np.asarray(bproj, dtype=np.float32)
    Wpre = np.asarray(Wpre, dtype=np.float32)
    Wpost = np.asarray(Wpost, dtype=np.float32)

    host = _host_prep(x, Wqkv, Wproj, bproj, Wpre, Wpost)
    if "nc" not in _NC_CACHE:
        _NC_CACHE["nc"] = _build_nc()
    nc = _NC_CACHE["nc"]

    shared = {k: host[k] for k in
              ("wqT", "wkT", "wvT", "wpT", "bdpre", "bdpostT", "bias")}
    in_maps = []
    for core in range(NCORES):
        m = dict(shared)
        m["xT"] = host["xT"][core * BPC:(core + 1) * BPC]
        in_maps.append(m)

    res = run_bass_kernel_spmd(nc, in_maps, core_ids=list(range(NCORES)))
    out = np.concatenate([np.asarray(r["y"]) for r in res.results], axis=0)
    return out.astype(np.float32)
